# revision 32
# baseline (speedup 1.0000x reference)
"""BREWA (bit-witness) attention on 8 TRN2 NeuronCores.

Sharding: core c = (batch b, head-group g) with b = c // 2, g = c % 2.
Each core computes its batch's attention for 8 of the 16 heads plus the
partial output projection over those heads' Wo columns; the host sums the
two partial projections per batch (the "all-reduce" is 2-way, done on CPU).

Per-core dataflow (all matmuls bf16, fp32 PSUM accumulate):
  xT[b] (host-transposed, bf16)  --PE-->  QT,KT [512,2048] (dims on partitions)
                                 --PE-->  V    [2048,520]  (seq on partitions,
                                                            65 cols/head: 64 V dims + ones)
  QT,KT --PE (block-diag wenc, K=128)--> enc psum --ACT tanh--> q_encT,k_encT
        [128, 2048] tiles: 4 heads x 32 bits on partitions, seq on free dim
  per (head-pair p, q-tile 512, k-tile 128):
    ST[k,q] via 2 row-tiled K=32 matmuls -> st psum [128, 1024]
    exp(ST/sqrt32) on ACT -> SBUF bf16   (softmax w/o max-sub: |scores|<=5.66)
    att[r] += V_aug[kt,h].T @ expST      (psum [65,512]; row 64 = sum_k exp = Z)
  normalize: DVE reciprocal(Z) -> GPSIMD partition_broadcast -> DVE mul -> c_T
  y = c_T.T @ WoT_g  (per-core partial, fp32 out)

Scheduling: DMA is split/ordered (wenc, wk, xt-nt0, wq, wv, xt-nt1..3, wo)
so the first QK matmul starts ~6us in; pair-0 qt-0's kt quarters are
interleaved with the QK/encoder ramp (ST only needs k_enc cols covered so
far); qk m2/m3 + encoder quad 1 drain as filler in pair0-qt1..pair1-qt0
windows; pairs 2/3 run qt-interleaved with the output projection chasing
one qt behind.
"""

import numpy as np
import ml_dtypes

import concourse.bacc as bacc
import concourse.bass as bass
import concourse.mybir as mybir
import concourse.tile as tile
from concourse.bass_utils import run_bass_kernel_spmd

B, N, D = 4, 2048, 1024
H, HD, MB = 16, 64, 32
NCORES = 8
HPG = 8              # heads per group (per core)
GD = HPG * HD        # 512 head dims per group
SCALE = float(1.0 / np.sqrt(MB))

bf16 = mybir.dt.bfloat16
f32 = mybir.dt.float32
BF = ml_dtypes.bfloat16
AF = mybir.ActivationFunctionType

KT_X = D // 128      # 8 contraction tiles over d_model
NT = N // 512        # 4 column tiles of 512 over sequence
MT_QK = GD // 128    # 4 partition tiles of QT/KT
NT128 = N // 128     # 16 row tiles of 128 over sequence
KT_C = GD // 128     # 4 contraction tiles over group head dims

TRACE = False        # set by test.py for profiling runs
TRACE_KW = {}
LAST_RESULTS = None
PHASE_LIMIT = "full"  # "qkv" | "attn" | "full" — for sim phase ablation
USE_FP8 = True       # fp8e4 DoubleRow scores matmul (2x PE on scores)
f8 = mybir.dt.float8e4


def build(reps=1):
    nc = bacc.Bacc("TRN2", target_bir_lowering=False, debug=False,
                   num_devices=NCORES)
    xt = nc.dram_tensor("xt", [D, N], bf16, kind="ExternalInput").ap()
    wq = nc.dram_tensor("wq", [D, GD], bf16, kind="ExternalInput").ap()
    wk = nc.dram_tensor("wk", [D, GD], bf16, kind="ExternalInput").ap()
    wv = nc.dram_tensor("wv", [D, GD], bf16, kind="ExternalInput").ap()
    wenc_cols = 8 * 64 if USE_FP8 else MT_QK * 64
    wenc = nc.dram_tensor("wenc", [128, wenc_cols], bf16,
                          kind="ExternalInput").ap()
    wo = nc.dram_tensor("wo", [GD, D], bf16, kind="ExternalInput").ap()
    y = nc.dram_tensor("y", [N, D], f32, kind="ExternalOutput").ap()

    with tile.TileContext(nc) as tc:
        with (
            tc.tile_pool(name="xtp", bufs=KT_X) as xt_pool,
            tc.tile_pool(name="wp", bufs=3 * KT_X) as w_pool,
            tc.tile_pool(name="wop", bufs=KT_C) as wo_pool,
            tc.tile_pool(name="wencp", bufs=1) as wenc_pool,
            tc.tile_pool(name="qkp", bufs=2 * MT_QK) as qk_pool,
            tc.tile_pool(name="encp", bufs=4) as enc_pool,
            tc.tile_pool(name="vp", bufs=NT128) as v_pool,
            tc.tile_pool(name="expp", bufs=12) as exp_pool,
            tc.tile_pool(name="ctp", bufs=KT_C) as ct_pool,
            tc.tile_pool(name="smallp", bufs=4) as small_pool,
            tc.tile_pool(name="yp", bufs=3) as y_pool,
            tc.tile_pool(name="stp", bufs=2, space="PSUM") as st_pool,
            tc.tile_pool(name="attp", bufs=2, space="PSUM") as att_pool,
            tc.tile_pool(name="bankp", bufs=2, space="PSUM") as bank_pool,
        ):
          for _rep in range(reps):
            # ---- input loads: emission order = SP queue order ----------------
            wenc_sb = wenc_pool.tile([128, wenc_cols], bf16, tag="wenc")
            nc.sync.dma_start(wenc_sb[:], wenc[:, :])

            def load_w(w_ap):
                tiles = []
                for k in range(KT_X):
                    t = w_pool.tile([128, GD], bf16, tag="w")
                    nc.sync.dma_start(t[:], w_ap[128 * k:128 * (k + 1), :])
                    tiles.append(t)
                return tiles

            wk_sb = load_w(wk)

            xt_sb = [xt_pool.tile([128, N], bf16, tag="xt", name=f"xt{k}")
                     for k in range(KT_X)]

            def load_xt_nt(nt):
                for k in range(KT_X):
                    nc.sync.dma_start(
                        xt_sb[k][:, 512 * nt:512 * (nt + 1)],
                        xt[128 * k:128 * (k + 1), 512 * nt:512 * (nt + 1)])

            load_xt_nt(0)
            wq_sb = load_w(wq)
            wv_sb = load_w(wv)
            for nt in range(1, NT):
                load_xt_nt(nt)
            wo_sb = []
            for k in range(KT_C):
                t = wo_pool.tile([128, D], bf16, tag="wo")
                nc.sync.dma_start(t[:], wo[128 * k:128 * (k + 1), :])
                wo_sb.append(t)

            # ---- QT / KT: [512 dims, 2048 seq], dims on partitions ----------
            qT_sb = [qk_pool.tile([128, N], bf16, tag="qk", name=f"qT{m}")
                     for m in range(MT_QK)]
            kT_sb = [qk_pool.tile([128, N], bf16, tag="qk", name=f"kT{m}")
                     for m in range(MT_QK)]
            enc_dt = f8 if USE_FP8 else bf16
            enc_w = 2 * N if USE_FP8 else N
            q_enc = [enc_pool.tile([128, enc_w], enc_dt, tag="enc",
                                   name=f"qenc{d}") for d in range(2)]
            k_enc = [enc_pool.tile([128, enc_w], enc_dt, tag="enc",
                                   name=f"kenc{d}") for d in range(2)]

            def qk_mtile_nt(wsb, t, mt, nt, name):
                ps = bank_pool.tile([128, 512], f32, tag="bank",
                                    name=f"ps_{name}_{nt}")
                for k in range(KT_X):
                    nc.tensor.matmul(
                        ps[:],
                        wsb[k][:, 128 * mt:128 * (mt + 1)],
                        xt_sb[k][:, 512 * nt:512 * (nt + 1)],
                        start=(k == 0), stop=(k == KT_X - 1),
                        skip_group_check=True,
                    )
                nc.vector.tensor_copy(t[:, 512 * nt:512 * (nt + 1)], ps[:])

            def encoder_nt_bf16(src, et, qd, nt, name):
                # block-diag wenc: one K=128 matmul covers an m-tile's 2 heads
                eps = bank_pool.tile([128, 512], f32, tag="bank",
                                     name=f"eps_{name}_{nt}")
                for j in range(2):
                    mt = 2 * qd + j
                    nc.tensor.matmul(
                        eps[64 * j:64 * (j + 1), :],
                        wenc_sb[:, 64 * mt:64 * (mt + 1)],
                        src[mt][:, 512 * nt:512 * (nt + 1)],
                        start=True, stop=True,
                        tile_position=(0, 64 * j),
                        skip_group_check=True,
                    )
                nc.scalar.activation(et[:, 512 * nt:512 * (nt + 1)],
                                     eps[:], AF.Tanh)

            def encoder_nt_fp8(src, et, qd, nt, name):
                # DR layout: head r (local in quad) at partitions 32r..32r+16,
                # bit-plane p at free offset 512p within a [128, 1024] psum
                # (2 banks). Stationary blocks are zero-padded so every out
                # partition is written. 4 matmuls: (pair j, plane p).
                # two [128,512] plane-halves from bank_pool so the tanh
                # never sits in the st_pool rotation gating the ST stream
                ev = et[:, :].rearrange("q (p n) -> q p n", p=2)
                for p in range(2):
                    eps = bank_pool.tile([128, 512], f32, tag="bank",
                                         name=f"eps_{name}_{nt}_{p}")
                    for j in range(2):
                        mt = 2 * qd + j
                        blk = 4 * qd + 2 * j + p
                        nc.tensor.matmul(
                            eps[64 * j:64 * (j + 1), :],
                            wenc_sb[:, 64 * blk:64 * (blk + 1)],
                            src[mt][:, 512 * nt:512 * (nt + 1)],
                            start=True, stop=True,
                            tile_position=(0, 64 * j),
                            skip_group_check=True,
                        )
                    nc.scalar.activation(
                        ev[:, p, 512 * nt:512 * (nt + 1)],
                        eps[:], AF.Tanh)

            encoder_nt = encoder_nt_fp8 if USE_FP8 else encoder_nt_bf16

            v_sb = [None] * NT128

            def ensure_v(nt):
                if v_sb[nt] is not None:
                    return v_sb[nt]
                t = v_pool.tile([128, HPG * 65], bf16, tag="v", name=f"v{nt}")
                ps = bank_pool.tile([128, 512], f32, tag="bank",
                                    name=f"ps_v{nt}")
                for k in range(KT_X):
                    nc.tensor.matmul(
                        ps[:],
                        xt_sb[k][:, 128 * nt:128 * (nt + 1)],
                        wv_sb[k][:],
                        start=(k == 0), stop=(k == KT_X - 1),
                        skip_group_check=True,
                    )
                vv = t[:, :].rearrange("p (h s) -> p h s", h=HPG)
                nc.vector.tensor_copy(
                    vv[:, :, 0:64],
                    ps[:, :].rearrange("p (h s) -> p h s", h=HPG),
                )
                nc.vector.memset(vv[:, :, 64:65], 1.0)
                v_sb[nt] = t
                return t

            # ---- c_T accumulator tiles: [512 head dims, 2048 seq] -----------
            ct_sb = [ct_pool.tile([128, N], bf16, tag="ct", name=f"ct{i}")
                     for i in range(KT_C)]

            def st_exp_kt(p, qt, kt):
                """ST -> exp for heads (2p, 2p+1) at one kt; returns ex."""
                qd = p // 2
                st = st_pool.tile([128, N // 2], f32, tag="st")
                for r in range(2):
                    a = 2 * (p % 2) + r
                    if USE_FP8:
                        kv = k_enc[qd][:, :].rearrange(
                            "q (j n) -> q j n", j=2)
                        qv = q_enc[qd][:, :].rearrange(
                            "q (j n) -> q j n", j=2)
                        nc.tensor.matmul(
                            st[:, 512 * r:512 * (r + 1)],
                            kv[32 * a:32 * a + 16, :,
                               128 * kt:128 * (kt + 1)],
                            qv[32 * a:32 * a + 16, :,
                               512 * qt:512 * (qt + 1)],
                            start=True, stop=True,
                            perf_mode=mybir.MatmulPerfMode.DoubleRow,
                            tile_position=(32 * a, 0),
                            skip_group_check=True,
                        )
                    else:
                        nc.tensor.matmul(
                            st[:, 512 * r:512 * (r + 1)],
                            k_enc[qd][32 * a:32 * (a + 1),
                                      128 * kt:128 * (kt + 1)],
                            q_enc[qd][32 * a:32 * (a + 1),
                                      512 * qt:512 * (qt + 1)],
                            start=True, stop=True,
                            tile_position=(32 * a, 0),
                            skip_group_check=True,
                        )
                ex = exp_pool.tile([128, N // 2], bf16, tag="exp")
                nc.scalar.activation(ex[:], st[:], AF.Exp, scale=SCALE)
                return ex

            def attv_kts(p, att, exs, kt_lo, kt_hi):
                for kt in range(kt_lo, kt_hi):
                    for r in range(2):
                        h = 2 * p + r
                        nc.tensor.matmul(
                            att[r][:],
                            ensure_v(kt)[:, 65 * h:65 * h + 65],
                            exs[kt][:, 512 * r:512 * (r + 1)],
                            start=(kt == 0), stop=(kt == NT128 - 1),
                            skip_group_check=True,
                        )

            def normalize(p, qt, att):
                for r in range(2):
                    h = 2 * p + r
                    recip = small_pool.tile([1, 512], f32, tag="recip")
                    nc.vector.reciprocal(recip[:], att[r][64:65, :])
                    bc = small_pool.tile([64, 512], f32, tag="bc")
                    nc.gpsimd.partition_broadcast(bc[:], recip[:])
                    u = 64 * (h % 2)
                    if u == 0:
                        nc.vector.tensor_mul(
                            ct_sb[h // 2][0:64, 512 * qt:512 * (qt + 1)],
                            att[r][0:64, :], bc[:])
                    else:
                        tmp = small_pool.tile([64, 512], bf16, tag="tmp")
                        nc.vector.tensor_mul(tmp[:], att[r][0:64, :], bc[:])
                        nc.sync.dma_start(
                            ct_sb[h // 2][64:128, 512 * qt:512 * (qt + 1)],
                            tmp[:])

            def new_att(p, qt):
                return [att_pool.tile([65, 512], f32, tag="att",
                                      name=f"att{p}_{qt}_{r}")
                        for r in range(2)]



            def out_proj_mt(mt):
                # y rows 128*mt .. 128*(mt+1): 2 out-dim halves
                for nt2 in range(2):
                    ps = bank_pool.tile([128, 512], f32, tag="bank",
                                        name=f"ps_y{mt}_{nt2}")
                    for k in range(KT_C):
                        nc.tensor.matmul(
                            ps[:],
                            ct_sb[k][:, 128 * mt:128 * (mt + 1)],
                            wo_sb[k][:, 512 * nt2:512 * (nt2 + 1)],
                            start=(k == 0), stop=(k == KT_C - 1),
                            skip_group_check=True,
                        )
                    yt = y_pool.tile([128, 512], f32, tag="y")
                    nc.vector.tensor_copy(yt[:], ps[:])
                    nc.sync.dma_start(
                        y[128 * mt:128 * (mt + 1),
                          512 * nt2:512 * (nt2 + 1)],
                        yt[:])

            def out_proj_qt(qt):
                for mt in range(4 * qt, 4 * qt + 4):
                    out_proj_mt(mt)

            # ---- emission script -------------------------------------------
            # ramp-min: everything pair0-qt0 kt0-3 needs (k_enc cols 0-511,
            # q_enc cols 0-511, V0/V1), then interleave pair0-qt0's kt
            # quarters with the remaining ramp nt slices.
            def kq_piece(which, mts, nt):
                # one deferred-prep piece: K or Q m-tile pair + encoder slice
                wsb, dst, enc_dst, enc_name = {
                    "k0": (wk_sb, kT_sb, k_enc[0], "kenc0"),
                    "q0": (wq_sb, qT_sb, q_enc[0], "qenc0"),
                    "k1": (wk_sb, kT_sb, k_enc[1], "kenc1"),
                    "q1": (wq_sb, qT_sb, q_enc[1], "qenc1"),
                }[which]
                qd = int(which[1])
                for mt in mts:
                    qk_mtile_nt(wsb, dst[mt], mt, nt, f"{which}m{mt}")
                encoder_nt(dst, enc_dst, qd, nt, enc_name)

            # ---- metronome + fill queue ------------------------------------
            # Each unit (pair, qt) emits a tight ST+exp stream (the metronome,
            # gating ACT); between kt quarters it drains fill pieces: the
            # previous unit's attV+normalize (front of queue), then deferred
            # prep (QK m-tiles/encoders, out-proj) from the back.
            from collections import deque
            fills = deque()

            def fill(n):
                for _ in range(n):
                    if fills:
                        fills.popleft()()

            def metronome(p, qt, nfill=2, direct=None):
                exs = []
                for q in range(NT):
                    for kt in range(4 * q, 4 * q + 4):
                        exs.append(st_exp_kt(p, qt, kt))
                    if direct is not None and q < NT - 1:
                        direct(q + 1)
                    fill(nfill)
                return exs

            def attv_norm_pieces(p, qt, att, exs):
                pieces = [
                    (lambda q=q: attv_kts(p, att, exs, 4 * q, 4 * q + 4))
                    for q in range(NT)
                ]
                pieces.append(lambda: normalize(p, qt, att))
                return pieces

            def out_piece(mt):
                return lambda: out_proj_mt(mt)

            # deferred prep, in first-use order (split into sub-2us pieces)
            for nt in range(1, NT):
                fills.append(lambda nt=nt: qk_mtile_nt(
                    wq_sb, qT_sb[0], 0, nt, "q0a"))
                fills.append(lambda nt=nt: (
                    qk_mtile_nt(wq_sb, qT_sb[1], 1, nt, "q0b"),
                    encoder_nt(qT_sb, q_enc[0], 0, nt, "qenc0")))
            for wsb_, dst_, enc_, en_ in (
                    (wk_sb, kT_sb, k_enc[1], "kenc1"),
                    (wq_sb, qT_sb, q_enc[1], "qenc1")):
                for nt in range(NT):
                    fills.append(
                        lambda w=wsb_, d=dst_, en=en_, nt=nt:
                        qk_mtile_nt(w, d[2], 2, nt, en + "a"))
                    fills.append(
                        lambda w=wsb_, d=dst_, e=enc_, en=en_, nt=nt: (
                            qk_mtile_nt(w, d[3], 3, nt, en + "b"),
                            encoder_nt(d, e, 1, nt, en)))

            # minimal ramp: quad-0 K and Q over nt0, V0/V1; unit (0,0)
            # interleaves the k0 nt>=1 slices directly (hard dep of its kt
            # sweep), no queue fills
            kq_piece("k0", (0, 1), 0)
            kq_piece("q0", (0, 1), 0)
            ensure_v(0)
            ensure_v(1)

            order = [(0, 0), (1, 0), (0, 1), (1, 1), (0, 2), (1, 2),
                     (0, 3), (1, 3), (2, 0), (3, 0), (2, 1), (3, 1),
                     (2, 2), (3, 2), (2, 3), (3, 3)]
            for i, (p, qt) in enumerate(order):
                att = new_att(p, qt)
                if i == 0:
                    exs = metronome(p, qt, nfill=0,
                                    direct=lambda nt: kq_piece(
                                        "k0", (0, 1), nt))
                else:
                    exs = metronome(p, qt, nfill=2)
                # previous unit's attV/normalize already queued; queue ours
                # at the front so they run in the next unit's windows
                pieces = attv_norm_pieces(p, qt, att, exs)
                if i == len(order) - 1:
                    for f in pieces:
                        f()
                else:
                    fills.extendleft(reversed(pieces))
                if p == 3:
                    # out-proj for qt becomes legal once pair3-qt normalize
                    # is queued; drains from the back of the queue
                    for mt in range(4 * qt, 4 * qt + 4):
                        fills.append(out_piece(mt))
                if PHASE_LIMIT == "qkv" and i == 0:
                    break
            if PHASE_LIMIT == "qkv":
                continue
            # drain whatever prep/out pieces remain
            while fills:
                fills.popleft()()
    nc.finalize()
    return nc


_nc_cache = None


def make_in_maps(inputs):
    x = np.asarray(inputs["x"], dtype=np.float32)
    Wq = np.asarray(inputs["Wq"], dtype=np.float32)
    Wk = np.asarray(inputs["Wk"], dtype=np.float32)
    Wv = np.asarray(inputs["Wv"], dtype=np.float32)
    We = np.asarray(inputs["W_enc"], dtype=np.float32)
    Wo = np.asarray(inputs["Wo"], dtype=np.float32)

    xts = [np.ascontiguousarray(x[b].T).astype(BF) for b in range(B)]
    in_maps = []
    for c in range(NCORES):
        b, g = divmod(c, 2)
        gs = g * GD
        we_g = We[g * HPG:(g + 1) * HPG]          # [8, 64, 32]
        if USE_FP8:
            # DR packing: block (qd, pair j, bit-plane p) at cols 64*blk;
            # rows 0-63 = head (4qd+2j) plane-p bits in cols 0-15,
            # rows 64-127 = head (4qd+2j+1) plane-p bits in cols 32-47;
            # zero elsewhere so every psum partition is written.
            we_blk = np.zeros((128, 8 * 64), np.float32)
            for qd in range(2):
                for j in range(2):
                    for p in range(2):
                        blk = 4 * qd + 2 * j + p
                        c = 64 * blk
                        we_blk[0:64, c:c + 16] = \
                            we_g[4 * qd + 2 * j][:, 16 * p:16 * (p + 1)]
                        we_blk[64:128, c + 32:c + 48] = \
                            we_g[4 * qd + 2 * j + 1][:, 16 * p:16 * (p + 1)]
        else:
            # block-diag packing: col block 64*t holds head pair (2t, 2t+1):
            # rows 0-63 = W_enc[2t] in cols 0-31, rows 64-127 = W_enc[2t+1]
            # in cols 32-63.
            we_blk = np.zeros((128, MT_QK * 64), np.float32)
            for t in range(MT_QK):
                we_blk[0:64, 64 * t:64 * t + 32] = we_g[2 * t]
                we_blk[64:128, 64 * t + 32:64 * t + 64] = we_g[2 * t + 1]
        in_maps.append({
            "xt": xts[b],
            "wq": np.ascontiguousarray(Wq[gs:gs + GD, :].T).astype(BF),
            "wk": np.ascontiguousarray(Wk[gs:gs + GD, :].T).astype(BF),
            "wv": np.ascontiguousarray(Wv[gs:gs + GD, :].T).astype(BF),
            "wenc": we_blk.astype(BF),
            "wo": np.ascontiguousarray(Wo[:, gs:gs + GD].T).astype(BF),
        })
    return in_maps


def kernel(**inputs):
    global _nc_cache, LAST_RESULTS
    if _nc_cache is None:
        _nc_cache = build()
    nc = _nc_cache
    in_maps = make_in_maps(inputs)

    res = run_bass_kernel_spmd(
        nc, in_maps, core_ids=list(range(NCORES)),
        trace=TRACE, **TRACE_KW)
    LAST_RESULTS = res

    out = np.empty((B, N, D), dtype=np.float32)
    for b in range(B):
        out[b] = res.results[2 * b]["y"] + res.results[2 * b + 1]["y"]
    return out


# revision 33
# speedup vs baseline: 1.1963x; 1.1963x over previous
"""BREWA (bit-witness) attention on 8 TRN2 NeuronCores.

Sharding: core c = (batch b, head-group g) with b = c // 2, g = c % 2.
Each core computes its batch's attention for 8 of the 16 heads plus the
partial output projection over those heads' Wo columns; the host sums the
two partial projections per batch (the "all-reduce" is 2-way, done on CPU).

Per-core dataflow (all matmuls bf16, fp32 PSUM accumulate):
  xT[b] (host-transposed, bf16)  --PE-->  QT,KT [512,2048] (dims on partitions)
                                 --PE-->  V    [2048,520]  (seq on partitions,
                                                            65 cols/head: 64 V dims + ones)
  QT,KT --PE (block-diag wenc, K=128)--> enc psum --ACT tanh--> q_encT,k_encT
        [128, 2048] tiles: 4 heads x 32 bits on partitions, seq on free dim
  per (head-pair p, q-tile 512, k-tile 128):
    ST[k,q] via 2 row-tiled K=32 matmuls -> st psum [128, 1024]
    exp(ST/sqrt32) on ACT -> SBUF bf16   (softmax w/o max-sub: |scores|<=5.66)
    att[r] += V_aug[kt,h].T @ expST      (psum [65,512]; row 64 = sum_k exp = Z)
  normalize: DVE reciprocal(Z) -> GPSIMD partition_broadcast -> DVE mul -> c_T
  y = c_T.T @ WoT_g  (per-core partial, fp32 out)

Scheduling: DMA is split/ordered (wenc, wk, xt-nt0, wq, wv, xt-nt1..3, wo)
so the first QK matmul starts ~6us in; pair-0 qt-0's kt quarters are
interleaved with the QK/encoder ramp (ST only needs k_enc cols covered so
far); qk m2/m3 + encoder quad 1 drain as filler in pair0-qt1..pair1-qt0
windows; pairs 2/3 run qt-interleaved with the output projection chasing
one qt behind.
"""

import numpy as np
import ml_dtypes

import concourse.bacc as bacc
import concourse.bass as bass
import concourse.mybir as mybir
import concourse.tile as tile
from concourse.bass_utils import run_bass_kernel_spmd

B, N, D = 4, 2048, 1024
H, HD, MB = 16, 64, 32
NCORES = 8
HPG = 8              # heads per group (per core)
GD = HPG * HD        # 512 head dims per group
SCALE = float(1.0 / np.sqrt(MB))

bf16 = mybir.dt.bfloat16
f32 = mybir.dt.float32
BF = ml_dtypes.bfloat16
AF = mybir.ActivationFunctionType

KT_X = D // 128      # 8 contraction tiles over d_model
NT = N // 512        # 4 column tiles of 512 over sequence
MT_QK = GD // 128    # 4 partition tiles of QT/KT
NT128 = N // 128     # 16 row tiles of 128 over sequence
KT_C = GD // 128     # 4 contraction tiles over group head dims

TRACE = False        # set by test.py for profiling runs
TRACE_KW = {}
LAST_RESULTS = None
PHASE_LIMIT = "full"  # "qkv" | "attn" | "full" — for sim phase ablation
# fp8e4 DoubleRow scores matmul (2x PE on scores): sims only ~2% faster
# than bf16 (ACT-bound attention phase) but costs 3.5x the rel-err margin
# (1.65e-2 vs 4.7e-3 against the 2e-2 gate) — keep bf16.
USE_FP8 = False
f8 = mybir.dt.float8e4


def build(reps=1):
    nc = bacc.Bacc("TRN2", target_bir_lowering=False, debug=False,
                   num_devices=NCORES)
    xt = nc.dram_tensor("xt", [D, N], bf16, kind="ExternalInput").ap()
    wq = nc.dram_tensor("wq", [D, GD], bf16, kind="ExternalInput").ap()
    wk = nc.dram_tensor("wk", [D, GD], bf16, kind="ExternalInput").ap()
    wv = nc.dram_tensor("wv", [D, GD], bf16, kind="ExternalInput").ap()
    wenc_cols = 8 * 64 if USE_FP8 else MT_QK * 64
    wenc = nc.dram_tensor("wenc", [128, wenc_cols], bf16,
                          kind="ExternalInput").ap()
    wo = nc.dram_tensor("wo", [GD, D], bf16, kind="ExternalInput").ap()
    y = nc.dram_tensor("y", [N, D], f32, kind="ExternalOutput").ap()

    with tile.TileContext(nc) as tc:
        with (
            tc.tile_pool(name="xtp", bufs=KT_X) as xt_pool,
            tc.tile_pool(name="wp", bufs=3 * KT_X) as w_pool,
            tc.tile_pool(name="wop", bufs=KT_C) as wo_pool,
            tc.tile_pool(name="wencp", bufs=1) as wenc_pool,
            tc.tile_pool(name="qkp", bufs=2 * MT_QK) as qk_pool,
            tc.tile_pool(name="encp", bufs=4) as enc_pool,
            tc.tile_pool(name="vp", bufs=NT128) as v_pool,
            tc.tile_pool(name="expp", bufs=12) as exp_pool,
            tc.tile_pool(name="ctp", bufs=KT_C) as ct_pool,
            tc.tile_pool(name="smallp", bufs=4) as small_pool,
            tc.tile_pool(name="yp", bufs=3) as y_pool,
            tc.tile_pool(name="stp", bufs=2, space="PSUM") as st_pool,
            tc.tile_pool(name="attp", bufs=2, space="PSUM") as att_pool,
            tc.tile_pool(name="bankp", bufs=2, space="PSUM") as bank_pool,
        ):
          for _rep in range(reps):
            # ---- input loads: emission order = SP queue order ----------------
            wenc_sb = wenc_pool.tile([128, wenc_cols], bf16, tag="wenc")
            nc.sync.dma_start(wenc_sb[:], wenc[:, :])

            def load_w(w_ap):
                tiles = []
                for k in range(KT_X):
                    t = w_pool.tile([128, GD], bf16, tag="w")
                    nc.sync.dma_start(t[:], w_ap[128 * k:128 * (k + 1), :])
                    tiles.append(t)
                return tiles

            wk_sb = load_w(wk)

            xt_sb = [xt_pool.tile([128, N], bf16, tag="xt", name=f"xt{k}")
                     for k in range(KT_X)]

            def load_xt_nt(nt):
                for k in range(KT_X):
                    nc.sync.dma_start(
                        xt_sb[k][:, 512 * nt:512 * (nt + 1)],
                        xt[128 * k:128 * (k + 1), 512 * nt:512 * (nt + 1)])

            load_xt_nt(0)
            wq_sb = load_w(wq)
            wv_sb = load_w(wv)
            for nt in range(1, NT):
                load_xt_nt(nt)
            wo_sb = []
            for k in range(KT_C):
                t = wo_pool.tile([128, D], bf16, tag="wo")
                nc.sync.dma_start(t[:], wo[128 * k:128 * (k + 1), :])
                wo_sb.append(t)

            # ---- QT / KT: [512 dims, 2048 seq], dims on partitions ----------
            qT_sb = [qk_pool.tile([128, N], bf16, tag="qk", name=f"qT{m}")
                     for m in range(MT_QK)]
            kT_sb = [qk_pool.tile([128, N], bf16, tag="qk", name=f"kT{m}")
                     for m in range(MT_QK)]
            enc_dt = f8 if USE_FP8 else bf16
            enc_w = 2 * N if USE_FP8 else N
            q_enc = [enc_pool.tile([128, enc_w], enc_dt, tag="enc",
                                   name=f"qenc{d}") for d in range(2)]
            k_enc = [enc_pool.tile([128, enc_w], enc_dt, tag="enc",
                                   name=f"kenc{d}") for d in range(2)]

            def qk_mtile_nt(wsb, t, mt, nt, name):
                ps = bank_pool.tile([128, 512], f32, tag="bank",
                                    name=f"ps_{name}_{nt}")
                for k in range(KT_X):
                    nc.tensor.matmul(
                        ps[:],
                        wsb[k][:, 128 * mt:128 * (mt + 1)],
                        xt_sb[k][:, 512 * nt:512 * (nt + 1)],
                        start=(k == 0), stop=(k == KT_X - 1),
                        skip_group_check=True,
                    )
                nc.vector.tensor_copy(t[:, 512 * nt:512 * (nt + 1)], ps[:])

            def encoder_nt_bf16(src, et, qd, nt, name):
                # block-diag wenc: one K=128 matmul covers an m-tile's 2 heads
                eps = bank_pool.tile([128, 512], f32, tag="bank",
                                     name=f"eps_{name}_{nt}")
                for j in range(2):
                    mt = 2 * qd + j
                    nc.tensor.matmul(
                        eps[64 * j:64 * (j + 1), :],
                        wenc_sb[:, 64 * mt:64 * (mt + 1)],
                        src[mt][:, 512 * nt:512 * (nt + 1)],
                        start=True, stop=True,
                        tile_position=(0, 64 * j),
                        skip_group_check=True,
                    )
                nc.scalar.activation(et[:, 512 * nt:512 * (nt + 1)],
                                     eps[:], AF.Tanh)

            def encoder_nt_fp8(src, et, qd, nt, name):
                # DR layout: head r (local in quad) at partitions 32r..32r+16,
                # bit-plane p at free offset 512p within a [128, 1024] psum
                # (2 banks). Stationary blocks are zero-padded so every out
                # partition is written. 4 matmuls: (pair j, plane p).
                # two [128,512] plane-halves from bank_pool so the tanh
                # never sits in the st_pool rotation gating the ST stream
                ev = et[:, :].rearrange("q (p n) -> q p n", p=2)
                for p in range(2):
                    eps = bank_pool.tile([128, 512], f32, tag="bank",
                                         name=f"eps_{name}_{nt}_{p}")
                    for j in range(2):
                        mt = 2 * qd + j
                        blk = 4 * qd + 2 * j + p
                        nc.tensor.matmul(
                            eps[64 * j:64 * (j + 1), :],
                            wenc_sb[:, 64 * blk:64 * (blk + 1)],
                            src[mt][:, 512 * nt:512 * (nt + 1)],
                            start=True, stop=True,
                            tile_position=(0, 64 * j),
                            skip_group_check=True,
                        )
                    nc.scalar.activation(
                        ev[:, p, 512 * nt:512 * (nt + 1)],
                        eps[:], AF.Tanh)

            encoder_nt = encoder_nt_fp8 if USE_FP8 else encoder_nt_bf16

            v_sb = [None] * NT128

            def ensure_v(nt):
                if v_sb[nt] is not None:
                    return v_sb[nt]
                t = v_pool.tile([128, HPG * 65], bf16, tag="v", name=f"v{nt}")
                ps = bank_pool.tile([128, 512], f32, tag="bank",
                                    name=f"ps_v{nt}")
                for k in range(KT_X):
                    nc.tensor.matmul(
                        ps[:],
                        xt_sb[k][:, 128 * nt:128 * (nt + 1)],
                        wv_sb[k][:],
                        start=(k == 0), stop=(k == KT_X - 1),
                        skip_group_check=True,
                    )
                vv = t[:, :].rearrange("p (h s) -> p h s", h=HPG)
                nc.vector.tensor_copy(
                    vv[:, :, 0:64],
                    ps[:, :].rearrange("p (h s) -> p h s", h=HPG),
                )
                nc.vector.memset(vv[:, :, 64:65], 1.0)
                v_sb[nt] = t
                return t

            # ---- c_T accumulator tiles: [512 head dims, 2048 seq] -----------
            ct_sb = [ct_pool.tile([128, N], bf16, tag="ct", name=f"ct{i}")
                     for i in range(KT_C)]

            def st_exp_kt(p, qt, kt):
                """ST -> exp for heads (2p, 2p+1) at one kt; returns ex."""
                qd = p // 2
                st = st_pool.tile([128, N // 2], f32, tag="st")
                for r in range(2):
                    a = 2 * (p % 2) + r
                    if USE_FP8:
                        kv = k_enc[qd][:, :].rearrange(
                            "q (j n) -> q j n", j=2)
                        qv = q_enc[qd][:, :].rearrange(
                            "q (j n) -> q j n", j=2)
                        nc.tensor.matmul(
                            st[:, 512 * r:512 * (r + 1)],
                            kv[32 * a:32 * a + 16, :,
                               128 * kt:128 * (kt + 1)],
                            qv[32 * a:32 * a + 16, :,
                               512 * qt:512 * (qt + 1)],
                            start=True, stop=True,
                            perf_mode=mybir.MatmulPerfMode.DoubleRow,
                            tile_position=(32 * a, 0),
                            skip_group_check=True,
                        )
                    else:
                        nc.tensor.matmul(
                            st[:, 512 * r:512 * (r + 1)],
                            k_enc[qd][32 * a:32 * (a + 1),
                                      128 * kt:128 * (kt + 1)],
                            q_enc[qd][32 * a:32 * (a + 1),
                                      512 * qt:512 * (qt + 1)],
                            start=True, stop=True,
                            tile_position=(32 * a, 0),
                            skip_group_check=True,
                        )
                ex = exp_pool.tile([128, N // 2], bf16, tag="exp")
                nc.scalar.activation(ex[:], st[:], AF.Exp, scale=SCALE)
                return ex

            def attv_kts(p, att, exs, kt_lo, kt_hi):
                for kt in range(kt_lo, kt_hi):
                    for r in range(2):
                        h = 2 * p + r
                        nc.tensor.matmul(
                            att[r][:],
                            ensure_v(kt)[:, 65 * h:65 * h + 65],
                            exs[kt][:, 512 * r:512 * (r + 1)],
                            start=(kt == 0), stop=(kt == NT128 - 1),
                            skip_group_check=True,
                        )

            def normalize(p, qt, att):
                for r in range(2):
                    h = 2 * p + r
                    recip = small_pool.tile([1, 512], f32, tag="recip")
                    nc.vector.reciprocal(recip[:], att[r][64:65, :])
                    bc = small_pool.tile([64, 512], f32, tag="bc")
                    nc.gpsimd.partition_broadcast(bc[:], recip[:])
                    u = 64 * (h % 2)
                    if u == 0:
                        nc.vector.tensor_mul(
                            ct_sb[h // 2][0:64, 512 * qt:512 * (qt + 1)],
                            att[r][0:64, :], bc[:])
                    else:
                        tmp = small_pool.tile([64, 512], bf16, tag="tmp")
                        nc.vector.tensor_mul(tmp[:], att[r][0:64, :], bc[:])
                        nc.sync.dma_start(
                            ct_sb[h // 2][64:128, 512 * qt:512 * (qt + 1)],
                            tmp[:])

            def new_att(p, qt):
                return [att_pool.tile([65, 512], f32, tag="att",
                                      name=f"att{p}_{qt}_{r}")
                        for r in range(2)]



            def out_proj_mt(mt):
                # y rows 128*mt .. 128*(mt+1): 2 out-dim halves
                for nt2 in range(2):
                    ps = bank_pool.tile([128, 512], f32, tag="bank",
                                        name=f"ps_y{mt}_{nt2}")
                    for k in range(KT_C):
                        nc.tensor.matmul(
                            ps[:],
                            ct_sb[k][:, 128 * mt:128 * (mt + 1)],
                            wo_sb[k][:, 512 * nt2:512 * (nt2 + 1)],
                            start=(k == 0), stop=(k == KT_C - 1),
                            skip_group_check=True,
                        )
                    yt = y_pool.tile([128, 512], f32, tag="y")
                    nc.vector.tensor_copy(yt[:], ps[:])
                    nc.sync.dma_start(
                        y[128 * mt:128 * (mt + 1),
                          512 * nt2:512 * (nt2 + 1)],
                        yt[:])

            def out_proj_qt(qt):
                for mt in range(4 * qt, 4 * qt + 4):
                    out_proj_mt(mt)

            # ---- emission script -------------------------------------------
            # ramp-min: everything pair0-qt0 kt0-3 needs (k_enc cols 0-511,
            # q_enc cols 0-511, V0/V1), then interleave pair0-qt0's kt
            # quarters with the remaining ramp nt slices.
            def kq_piece(which, mts, nt):
                # one deferred-prep piece: K or Q m-tile pair + encoder slice
                wsb, dst, enc_dst, enc_name = {
                    "k0": (wk_sb, kT_sb, k_enc[0], "kenc0"),
                    "q0": (wq_sb, qT_sb, q_enc[0], "qenc0"),
                    "k1": (wk_sb, kT_sb, k_enc[1], "kenc1"),
                    "q1": (wq_sb, qT_sb, q_enc[1], "qenc1"),
                }[which]
                qd = int(which[1])
                for mt in mts:
                    qk_mtile_nt(wsb, dst[mt], mt, nt, f"{which}m{mt}")
                encoder_nt(dst, enc_dst, qd, nt, enc_name)

            # ---- metronome + fill queue ------------------------------------
            # Each unit (pair, qt) emits a tight ST+exp stream (the metronome,
            # gating ACT); between kt quarters it drains fill pieces: the
            # previous unit's attV+normalize (front of queue), then deferred
            # prep (QK m-tiles/encoders, out-proj) from the back.
            from collections import deque
            fills = deque()

            def fill(n):
                for _ in range(n):
                    if fills:
                        fills.popleft()()

            def metronome(p, qt, nfill=2, direct=None):
                exs = []
                for q in range(NT):
                    for kt in range(4 * q, 4 * q + 4):
                        exs.append(st_exp_kt(p, qt, kt))
                    if direct is not None and q < NT - 1:
                        direct(q + 1)
                    fill(nfill)
                return exs

            def attv_norm_pieces(p, qt, att, exs):
                pieces = [
                    (lambda q=q: attv_kts(p, att, exs, 4 * q, 4 * q + 4))
                    for q in range(NT)
                ]
                pieces.append(lambda: normalize(p, qt, att))
                return pieces

            def out_piece(mt):
                return lambda: out_proj_mt(mt)

            # deferred prep, in first-use order (split into sub-2us pieces)
            for nt in range(1, NT):
                fills.append(lambda nt=nt: qk_mtile_nt(
                    wq_sb, qT_sb[0], 0, nt, "q0a"))
                fills.append(lambda nt=nt: (
                    qk_mtile_nt(wq_sb, qT_sb[1], 1, nt, "q0b"),
                    encoder_nt(qT_sb, q_enc[0], 0, nt, "qenc0")))
            for wsb_, dst_, enc_, en_ in (
                    (wk_sb, kT_sb, k_enc[1], "kenc1"),
                    (wq_sb, qT_sb, q_enc[1], "qenc1")):
                for nt in range(NT):
                    fills.append(
                        lambda w=wsb_, d=dst_, en=en_, nt=nt:
                        qk_mtile_nt(w, d[2], 2, nt, en + "a"))
                    fills.append(
                        lambda w=wsb_, d=dst_, e=enc_, en=en_, nt=nt: (
                            qk_mtile_nt(w, d[3], 3, nt, en + "b"),
                            encoder_nt(d, e, 1, nt, en)))

            # minimal ramp: quad-0 K and Q over nt0, V0/V1; unit (0,0)
            # interleaves the k0 nt>=1 slices directly (hard dep of its kt
            # sweep), no queue fills
            kq_piece("k0", (0, 1), 0)
            kq_piece("q0", (0, 1), 0)
            ensure_v(0)
            ensure_v(1)

            order = [(0, 0), (1, 0), (0, 1), (1, 1), (0, 2), (1, 2),
                     (0, 3), (1, 3), (2, 0), (3, 0), (2, 1), (3, 1),
                     (2, 2), (3, 2), (2, 3), (3, 3)]
            for i, (p, qt) in enumerate(order):
                att = new_att(p, qt)
                if i == 0:
                    exs = metronome(p, qt, nfill=0,
                                    direct=lambda nt: kq_piece(
                                        "k0", (0, 1), nt))
                else:
                    exs = metronome(p, qt, nfill=2)
                # previous unit's attV/normalize already queued; queue ours
                # at the front so they run in the next unit's windows
                pieces = attv_norm_pieces(p, qt, att, exs)
                if i == len(order) - 1:
                    for f in pieces:
                        f()
                else:
                    fills.extendleft(reversed(pieces))
                if p == 3:
                    # out-proj for qt becomes legal once pair3-qt normalize
                    # is queued; drains from the back of the queue
                    for mt in range(4 * qt, 4 * qt + 4):
                        fills.append(out_piece(mt))
                if PHASE_LIMIT == "qkv" and i == 0:
                    break
            if PHASE_LIMIT == "qkv":
                continue
            # drain whatever prep/out pieces remain
            while fills:
                fills.popleft()()
    nc.finalize()
    return nc


_nc_cache = None


def make_in_maps(inputs):
    x = np.asarray(inputs["x"], dtype=np.float32)
    Wq = np.asarray(inputs["Wq"], dtype=np.float32)
    Wk = np.asarray(inputs["Wk"], dtype=np.float32)
    Wv = np.asarray(inputs["Wv"], dtype=np.float32)
    We = np.asarray(inputs["W_enc"], dtype=np.float32)
    Wo = np.asarray(inputs["Wo"], dtype=np.float32)

    xts = [np.ascontiguousarray(x[b].T).astype(BF) for b in range(B)]
    in_maps = []
    for c in range(NCORES):
        b, g = divmod(c, 2)
        gs = g * GD
        we_g = We[g * HPG:(g + 1) * HPG]          # [8, 64, 32]
        if USE_FP8:
            # DR packing: block (qd, pair j, bit-plane p) at cols 64*blk;
            # rows 0-63 = head (4qd+2j) plane-p bits in cols 0-15,
            # rows 64-127 = head (4qd+2j+1) plane-p bits in cols 32-47;
            # zero elsewhere so every psum partition is written.
            we_blk = np.zeros((128, 8 * 64), np.float32)
            for qd in range(2):
                for j in range(2):
                    for p in range(2):
                        blk = 4 * qd + 2 * j + p
                        c = 64 * blk
                        we_blk[0:64, c:c + 16] = \
                            we_g[4 * qd + 2 * j][:, 16 * p:16 * (p + 1)]
                        we_blk[64:128, c + 32:c + 48] = \
                            we_g[4 * qd + 2 * j + 1][:, 16 * p:16 * (p + 1)]
        else:
            # block-diag packing: col block 64*t holds head pair (2t, 2t+1):
            # rows 0-63 = W_enc[2t] in cols 0-31, rows 64-127 = W_enc[2t+1]
            # in cols 32-63.
            we_blk = np.zeros((128, MT_QK * 64), np.float32)
            for t in range(MT_QK):
                we_blk[0:64, 64 * t:64 * t + 32] = we_g[2 * t]
                we_blk[64:128, 64 * t + 32:64 * t + 64] = we_g[2 * t + 1]
        in_maps.append({
            "xt": xts[b],
            "wq": np.ascontiguousarray(Wq[gs:gs + GD, :].T).astype(BF),
            "wk": np.ascontiguousarray(Wk[gs:gs + GD, :].T).astype(BF),
            "wv": np.ascontiguousarray(Wv[gs:gs + GD, :].T).astype(BF),
            "wenc": we_blk.astype(BF),
            "wo": np.ascontiguousarray(Wo[:, gs:gs + GD].T).astype(BF),
        })
    return in_maps


def kernel(**inputs):
    global _nc_cache, LAST_RESULTS
    if _nc_cache is None:
        _nc_cache = build()
    nc = _nc_cache
    in_maps = make_in_maps(inputs)

    res = run_bass_kernel_spmd(
        nc, in_maps, core_ids=list(range(NCORES)),
        trace=TRACE, **TRACE_KW)
    LAST_RESULTS = res

    out = np.empty((B, N, D), dtype=np.float32)
    for b in range(B):
        out[b] = res.results[2 * b]["y"] + res.results[2 * b + 1]["y"]
    return out
